# revision 2
# baseline (speedup 1.0000x reference)
"""Causal self-attention (quirky-reshape variant) on 8 TRN2 NeuronCores, v3.

Head h's Q/K/V come from rows [256h, 256h+256) of the [4096,1024] projection
output reinterpreted as [4096,64]; output rows [256h, 256h+256) depend only on
head h.  Core i: heads 2i,2i+1, x rows [512i, 512i+512), no collectives.

Layouts (hot matmul operands contiguous):
  - Q/outT free order u' = 512*(r//32) + 32*c + (r%32)  ("block-c32"):
    score moving operands and PV outputs are contiguous 512-wide blocks.
  - K s-ordered (strided PSUM copies); scores stationary contiguous [64,128].
  - vsb per head: [128, 128*T] = [V(64) | ones(64)] per tile: P@V replicates
    softmax denominators onto PSUM partitions 64:128.
  - Projection order V -> Q -> K; K-RoPE streams in 512-col s-chunks so
    attention block b only needs chunk b.
  - DMA staged via tiny DVE "touch" ops so early-phase weights get full HBM bw.
  - engine split: ACT = exp + copy share; DVE = copies/swaps/normalize;
    GPSIMD = RoPE muls.
"""

import os

os.environ.setdefault("JAX_PLATFORMS", "cpu")

import numpy as np
import ml_dtypes

D = 1024          # d_model
H = 16            # heads
DK = 64           # head dim
S = 4096          # seq len
RPC = 512         # x rows per core
HPC = 2           # heads per core
NT_SK = 32        # sk tiles of 128 per head
NC_N = 8
ROPE_THETA = 10000.0

# Schraudolph fast-exp (bf16 bits): bits = round((s*0.125)*A16 + B16)
FEXP_A16 = 0.125 * 128.0 / np.log(2.0)          # 23.0831
FEXP_C16 = 366393.0 / 65536.0
FEXP_B16 = 127.0 * 128.0 - FEXP_C16
# fraction of fully-below-diagonal chunks offloaded to DVE (0 = all on ACT)
FEXP_NUM = int(os.environ.get("FEXP_NUM", "0"))
FEXP_DEN = int(os.environ.get("FEXP_DEN", "2"))

_CACHE = {}


def _deint_perm():
    """o' -> o source index: within each 64-block, evens first then odds."""
    d_order = list(range(0, DK, 2)) + list(range(1, DK, 2))
    perm = np.zeros(D, dtype=np.int64)
    for c in range(H):
        for dp, d in enumerate(d_order):
            perm[c * DK + dp] = c * DK + d
    return perm


def _rope_tables(s_of_col):
    bf = ml_dtypes.bfloat16
    j = np.arange(0, DK, 2, dtype=np.float64) / DK
    inv_freq = 1.0 / (ROPE_THETA ** j)                      # [32]
    ang = np.outer(inv_freq, s_of_col)                      # [32, S]
    cs1 = np.concatenate([np.cos(ang)] * 4, 0).astype(bf)
    cs2 = np.concatenate([-np.sin(ang), np.sin(ang)] * 2, 0).astype(bf)
    return cs1, cs2


def _host_arrays(Wq, Wk, Wv, Wo):
    bf = ml_dtypes.bfloat16
    perm = _deint_perm()
    wqT = np.ascontiguousarray(Wq[perm, :].T).astype(bf)    # [in, o'] deint
    wkT = np.ascontiguousarray(Wk[perm, :].T).astype(bf)
    wvT = np.ascontiguousarray(Wv.T).astype(bf)             # [in, o] natural
    woT = np.ascontiguousarray(Wo.T).astype(bf)             # [o_c, o_out]

    u = np.arange(S)
    s_k = u                                                 # s-ordered (K)
    s_q = 16 * (32 * (u // 512) + (u % 32)) + (u % 512) // 32  # block-c32 (Q)
    csk1, csk2 = _rope_tables(s_k)
    csq1, csq2 = _rope_tables(s_q)

    # tri mask: row p = sk local (plain), col = 8*cq + r8
    p = np.arange(128)
    col = np.arange(128)
    cq, r8 = col // 8, col % 8
    sq_loc = 16 * r8 + cq
    tri = np.where(p[:, None] <= sq_loc[None, :], 0.0, -1e30
                   ).astype(np.float32)
    return wqT, wkT, wvT, woT, csq1, csq2, csk1, csk2, tri


def _build_program(dbg=False):
    import concourse.bass as bass
    import concourse.tile as tile
    from concourse import bacc, mybir

    f32 = mybir.dt.float32
    bf16 = mybir.dt.bfloat16
    i16 = mybir.dt.int16
    EXP = mybir.ActivationFunctionType.Exp
    CPY = mybir.ActivationFunctionType.Copy
    MULT = mybir.AluOpType.mult
    ADD = mybir.AluOpType.add

    nc = bacc.Bacc("TRN2", target_bir_lowering=False, debug=False,
                   num_devices=NC_N)

    xT = nc.dram_tensor("xT", [D, RPC], bf16, kind="ExternalInput").ap()
    wq = nc.dram_tensor("wqT", [D, D], bf16, kind="ExternalInput").ap()
    wk = nc.dram_tensor("wkT", [D, D], bf16, kind="ExternalInput").ap()
    wv = nc.dram_tensor("wvT", [D, D], bf16, kind="ExternalInput").ap()
    wo = nc.dram_tensor("woT", [D, D], bf16, kind="ExternalInput").ap()
    cq1d = nc.dram_tensor("csq1", [128, S], bf16, kind="ExternalInput").ap()
    cq2d = nc.dram_tensor("csq2", [128, S], bf16, kind="ExternalInput").ap()
    ck1d = nc.dram_tensor("csk1", [128, S], bf16, kind="ExternalInput").ap()
    ck2d = nc.dram_tensor("csk2", [128, S], bf16, kind="ExternalInput").ap()
    trid = nc.dram_tensor("tri", [128, 128], f32, kind="ExternalInput").ap()
    y = nc.dram_tensor("y", [RPC, D], f32, kind="ExternalOutput").ap()
    vfd = nc.dram_tensor("vflat_scratch", [RPC, D], bf16).ap()
    dbg_aps = {}
    if dbg:
        for nm, shp, dt in [
            ("dbg_qrot", [128, S], bf16), ("dbg_krot", [128, S], bf16),
            ("dbg_vsb0", [128, 128 * NT_SK], bf16),
            ("dbg_outT0", [64, S], bf16), ("dbg_outT1", [64, S], bf16),
            ("dbg_den0", [128, 512], f32),
        ]:
            dbg_aps[nm] = nc.dram_tensor(nm, shp, dt, kind="ExternalOutput").ap()

    with tile.TileContext(nc) as tc:
        with (
            tc.tile_pool(name="big", bufs=3) as big,     # wv/wq/wk -> outT/y_sb
            tc.tile_pool(name="wo", bufs=1) as wop,
            tc.tile_pool(name="xp", bufs=1) as xp,
            tc.tile_pool(name="qk", bufs=2) as qkp,
            tc.tile_pool(name="rope", bufs=4) as ropep,
            tc.tile_pool(name="sw", bufs=3) as swp,
            tc.tile_pool(name="vf", bufs=1) as vfp,
            tc.tile_pool(name="vsb", bufs=2) as vsbp,
            tc.tile_pool(name="mask", bufs=1) as maskp,
            tc.tile_pool(name="pp", bufs=3) as ppool,    # exp'd P chunks
            tc.tile_pool(name="nrm", bufs=2) as nrmp,    # normalize staging
            tc.tile_pool(name="ct", bufs=2) as ctp,
        ):
            # ---------------- phase 0: staged loads ----------------
            xsb = xp.tile([128, 8 * RPC], bf16, tag="x")      # [p, kt*512+r]
            wv_sb = big.tile([128, 8 * D], bf16, tag="big", name="w_wv")
            wq_sb = big.tile([128, 8 * D], bf16, tag="big", name="w_wq")
            wk_sb = big.tile([128, 8 * D], bf16, tag="big", name="w_wk")
            wo_sb = wop.tile([128, 8 * D], bf16, tag="wo", name="w_wo")
            ck1_sb = ropep.tile([128, S], bf16, tag="rope", name="ck1")
            ck2_sb = ropep.tile([128, S], bf16, tag="rope", name="ck2")
            cq1_sb = ropep.tile([128, S], bf16, tag="rope", name="cq1")
            cq2_sb = ropep.tile([128, S], bf16, tag="rope", name="cq2")
            tri_sb = maskp.tile([128, 128], f32, tag="mask")

            def ld(dst_tile, src):
                nc.sync.dma_start(
                    dst_tile[:].rearrange("p (kt o) -> p kt o", kt=8),
                    src.rearrange("(kt p) o -> p kt o", p=128))

            def touch(dst_tile, src_tile):
                # gate dst_tile's DMA on src_tile's DMA completion: read the
                # last elements of every block of src, write dst corners
                sv = src_tile[0:1, :].rearrange("p (kt o) -> p kt o", kt=8)
                dv = dst_tile[0:1, :].rearrange("p (kt o) -> p kt o", kt=8)
                w = sv.shape[-1]
                nc.vector.tensor_copy(dv[:, :, 0:2], sv[:, :, w - 2:w])

            # stage A: x + wv
            nc.sync.dma_start(xsb[:].rearrange("p (kt r) -> p kt r", kt=8),
                              xT.rearrange("(kt p) r -> p kt r", p=128))
            ld(wv_sb, wv)
            # stage B: wq + csq (gated on A)
            touch(wq_sb, xsb)
            touch(cq1_sb, wv_sb)
            touch(cq2_sb, wv_sb)
            ld(wq_sb, wq)
            for half in range(2):
                nc.sync.dma_start(cq1_sb[:, 2048 * half: 2048 * (half + 1)],
                                  cq1d[:, 2048 * half: 2048 * (half + 1)])
                nc.sync.dma_start(cq2_sb[:, 2048 * half: 2048 * (half + 1)],
                                  cq2d[:, 2048 * half: 2048 * (half + 1)])
            # stage C: wk + csk + tri (gated on B)
            touch(wk_sb, wq_sb)
            touch(ck1_sb, wq_sb)
            touch(ck2_sb, wq_sb)
            nc.vector.tensor_copy(tri_sb[0:1, 0:2], wq_sb[0:1, 0:2])
            ld(wk_sb, wk)
            for half in range(2):
                nc.sync.dma_start(ck1_sb[:, 2048 * half: 2048 * (half + 1)],
                                  ck1d[:, 2048 * half: 2048 * (half + 1)])
                nc.sync.dma_start(ck2_sb[:, 2048 * half: 2048 * (half + 1)],
                                  ck2d[:, 2048 * half: 2048 * (half + 1)])
            nc.sync.dma_start(tri_sb[:], trid[:])
            # stage D: wo (gated on C)
            touch(wo_sb, wk_sb)
            ld(wo_sb, wo)

            warm = maskp.tile([128, 640], bf16, tag="warm", name="warm")
            nc.vector.memset(warm[:], 1.0)

            # vsb tiles memset to 1.0 early (ones block survives the V gather)
            vsbs = []
            for h in range(HPC):
                t = vsbp.tile([128, 128 * NT_SK], bf16, tag="vsb",
                              name=f"vsb{h}")
                nc.gpsimd.memset(t[:], 1.0)
                vsbs.append(t)

            # ---------------- phase 1a: V projection (kt-outer) ----------------
            vflat = vfp.tile([128, 4 * D], bf16, tag="vf")
            with tc.tile_pool(name="psv", bufs=8, space="PSUM") as psvp:
                vps = [psvp.tile([128, 512], f32, tag="vps", name=f"vps{i}")
                       for i in range(8)]
                for kt in range(8):
                    for idx in range(8):
                        rt, ob = idx // 2, idx % 2
                        nc.tensor.matmul(
                            vps[idx][:],
                            xsb[:, kt * RPC + rt * 128: kt * RPC + rt * 128 + 128],
                            wv_sb[:, kt * D + ob * 512: kt * D + ob * 512 + 512],
                            start=(kt == 0), stop=(kt == 7),
                        )
                for idx in range(8):
                    rt, ob = idx // 2, idx % 2
                    nc.vector.tensor_copy(
                        vflat[:, rt * D + ob * 512: rt * D + ob * 512 + 512],
                        vps[idx][:])

            # V reshape through DRAM; vsb row p = 16a + c holds V[128T+16a+c]
            vfl = vflat[:].rearrange("p (rt o) -> p rt o", rt=4)
            vfdv = vfd.rearrange("(rt p) o -> p rt o", p=128)
            for rt in range(4):
                nc.sync.dma_start(vfdv[:, rt, :], vfl[:, rt, :])
            vld = vfd.rearrange("(h T a) (c d) -> h a c T d", h=2, T=NT_SK,
                                a=8, c=16, d=DK)
            for h in range(HPC):
                dstv = vsbs[h][:].rearrange("(a c) (T d) -> a c T d", a=8,
                                            c=16, T=NT_SK, d=128)
                for a in range(8):
                    nc.sync.dma_start(dstv[a, :, :, 0:DK], vld[h, a])

            # ---------------- phase 1b: Q then K projections ----------------
            qraw = qkp.tile([128, S], bf16, tag="qk", name="qraw")
            kraw = qkp.tile([128, S], bf16, tag="qk", name="kraw")
            sw_q = swp.tile([128, S], bf16, tag="sw", name="sw_q")
            sw_k = swp.tile([128, S], bf16, tag="sw", name="sw_k")
            qv4 = qraw[:].rearrange("p (blk c r) -> p blk c r", blk=8, c=16,
                                    r=32)

            def rope_chunk(raw, sw, c1, c2, u0, w):
                """RoPE cols [u0,u0+w): swaps+add on DVE, muls on GPSIMD."""
                for (d0, s0) in ((0, 32), (32, 0), (64, 96), (96, 64)):
                    nc.vector.tensor_copy(sw[d0:d0 + 32, u0:u0 + w],
                                          raw[s0:s0 + 32, u0:u0 + w])
                nc.gpsimd.tensor_mul(sw[:, u0:u0 + w], sw[:, u0:u0 + w],
                                     c2[:, u0:u0 + w])
                nc.vector.tensor_mul(raw[:, u0:u0 + w], raw[:, u0:u0 + w],
                                     c1[:, u0:u0 + w])
                nc.gpsimd.tensor_add(raw[:, u0:u0 + w], raw[:, u0:u0 + w],
                                     sw[:, u0:u0 + w])

            with tc.tile_pool(name="psp", bufs=4, space="PSUM") as psp:
                # Q: block-c32 layout; copies are 3D (64B runs)
                for ot in range(8):
                    pq = psp.tile([128, RPC], f32, tag="ps")
                    for kt in range(8):
                        nc.tensor.matmul(
                            pq[:],
                            wq_sb[:, kt * D + ot * 128: kt * D + ot * 128 + 128],
                            xsb[:, kt * RPC: (kt + 1) * RPC],
                            start=(kt == 0), stop=(kt == 7),
                        )
                    c0 = 2 * ot
                    sv = pq[:].rearrange("p (half blk r) -> p half blk r",
                                         half=2, blk=8, r=32)
                    nc.scalar.activation(qv4[0:64, :, c0, :],
                                         sv[0:64, 0, :, :], CPY)
                    nc.vector.tensor_copy(qv4[64:128, :, c0, :],
                                          sv[0:64, 1, :, :])
                    nc.vector.tensor_copy(qv4[0:64, :, c0 + 1, :],
                                          sv[64:128, 0, :, :])
                    nc.scalar.activation(qv4[64:128, :, c0 + 1, :],
                                         sv[64:128, 1, :, :], CPY)
                # K: s-ordered storage, strided dst copies (stride 16)
                kv_raw = kraw[:].rearrange("p (r c) -> p r c", c=16)
                for ot in range(8):
                    pk = psp.tile([128, RPC], f32, tag="ps")
                    for kt in range(8):
                        nc.tensor.matmul(
                            pk[:],
                            wk_sb[:, kt * D + ot * 128: kt * D + ot * 128 + 128],
                            xsb[:, kt * RPC: (kt + 1) * RPC],
                            start=(kt == 0), stop=(kt == 7),
                        )
                    c0 = 2 * ot
                    nc.scalar.activation(kv_raw[0:64, 0:256, c0],
                                         pk[0:64, 0:256], CPY)
                    nc.scalar.activation(kv_raw[64:128, 0:256, c0 + 1],
                                         pk[64:128, 256:512], CPY)
                    if ot % 2 == 0:
                        nc.scalar.activation(kv_raw[64:128, 0:256, c0],
                                             pk[0:64, 256:512], CPY)
                    else:
                        nc.vector.tensor_copy(kv_raw[64:128, 0:256, c0],
                                              pk[0:64, 256:512])
                    nc.vector.tensor_copy(kv_raw[0:64, 0:256, c0 + 1],
                                          pk[64:128, 0:256])
                # K RoPE: 512-col s-chunks stream ahead of attention blocks
                for g in range(8):
                    rope_chunk(kraw, sw_k, ck1_sb, ck2_sb, 512 * g, 512)

            qrot, krot = qraw, kraw
            tri3 = tri_sb[:].rearrange("p (c r) -> p c r", r=8)
            if dbg:
                nc.sync.dma_start(dbg_aps["dbg_qrot"][:], qrot[:])
                nc.sync.dma_start(dbg_aps["dbg_krot"][:], krot[:])
                nc.sync.dma_start(dbg_aps["dbg_vsb0"][:], vsbs[0][:])

            # ---------------- phase 2: attention ----------------
            outTs = []
            for h in range(HPC):
                outTs.append(big.tile([64, S], bf16, tag="big",
                                      name=f"outT{h}"))

            cts = {}

            def emit_ct(h, rt):
                g = 2 * h + rt
                ct = ctp.tile([128, 8 * 128], bf16, tag="ct", name=f"ct{g}")
                cts[g] = ct
                ov4 = outTs[h][:].rearrange("p (blk c r) -> p blk c r",
                                            blk=8, c=16, r=32)
                for tp in range(8):
                    for par in range(2):
                        j = 2 * tp + par
                        src = ov4[:, 4 * rt: 4 * rt + 4, j, :]
                        dst = ct[64 * par: 64 * par + 64,
                                 128 * tp: 128 * tp + 128].rearrange(
                            "p (ib r) -> p ib r", r=32)
                        nc.vector.tensor_copy(dst, src)

            def emit_yproj_ob(h, rt, ob):
                g = 2 * h + rt
                ct = cts[g]
                py = psoutp.tile([128, 512], f32, tag="out",
                                 name=f"py{g}_{ob}")
                for tp in range(8):
                    nc.tensor.matmul(
                        py[:],
                        ct[:, 128 * tp: 128 * tp + 128],
                        wo_sb[:, tp * D + ob * 512: tp * D + ob * 512 + 512],
                        start=(tp == 0), stop=(tp == 7),
                    )
                nc.scalar.activation(
                    y_sb[:, g * D + ob * 512: g * D + ob * 512 + 512],
                    py[:], CPY)
                if ob == 1:
                    nc.sync.dma_start(ydv[:, g, :], y_sb[:].rearrange(
                        "p (g o) -> p g o", g=4)[:, g, :])

            def emit_yproj(h, rt):
                emit_yproj_ob(h, rt, 0)
                emit_yproj_ob(h, rt, 1)

            y_sb = None
            ydv = y.rearrange("(g p) o -> p g o", p=128)
            with (
                tc.tile_pool(name="pssc", bufs=2, space="PSUM") as pssc,
                tc.tile_pool(name="psout", bufs=2, space="PSUM") as psoutp,
            ):
                fexp_ctr = 0
                pending = None   # (chunk, pch, outp, opv3, b, nt)

                def emit_pv(job):
                    chunk, pch, joutp, jopv3, jb, jnt = job
                    for idx, (t, h) in enumerate(chunk):
                        jd = t - 4 * jb
                        if jd < 0:
                            nc.tensor.matmul(
                                joutp[h][:],
                                vsbs[h][:, 128 * t: 128 * t + 128],
                                pch[:, 512 * idx: 512 * (idx + 1)],
                                start=(t == 0), stop=(t == jnt - 1),
                            )
                        else:
                            rmin = 8 * jd
                            pcv3 = pch[:, 512 * idx: 512 * (idx + 1)]\
                                .rearrange("p (c r) -> p c r", r=32)
                            nc.tensor.matmul(
                                jopv3[h][:, :, rmin:32],
                                vsbs[h][:, 128 * t: 128 * t + 128],
                                pcv3[:, :, rmin:32],
                                start=(t == 0), stop=(t == jnt - 1),
                            )

                for b in range(8):
                    if b == 0:
                        rope_chunk(kraw, sw_k, ck1_sb, ck2_sb, 0, 512)
                        rope_chunk(kraw, sw_k, ck1_sb, ck2_sb, 512, 512)
                    nt = 4 * (b + 1)
                    slots = [(t, h) for t in range(nt) for h in range(HPC)]
                    outp = [psoutp.tile([128, 512], f32, tag="out",
                                        name=f"outp{b}_{h}")
                            for h in range(HPC)]
                    opv3 = [outp[h][:].rearrange("p (c r) -> p c r", r=32)
                            for h in range(HPC)]
                    chunks = [slots[i:i + 3] for i in range(0, len(slots), 3)]
                    mid = max(1, len(chunks) // 2)
                    for ci, chunk in enumerate(chunks):
                        if ci == mid and 0 < b < 7:
                            rope_chunk(kraw, sw_k, ck1_sb, ck2_sb,
                                       512 * (b + 1), 512)
                        if b == 6 and ci in (3, 10):
                            emit_yproj_ob(0, 0, 0 if ci == 3 else 1)
                        elif b == 7 and ci in (3, 12):
                            emit_yproj_ob(1, 0, 0 if ci == 3 else 1)
                        ps = pssc.tile([128, 1536], f32, tag="sc")
                        pch = ppool.tile([128, 1536], bf16, tag="pp")
                        for idx, (t, h) in enumerate(chunk):
                            jd = t - 4 * b
                            if jd < 0:
                                nc.tensor.matmul(
                                    ps[:, 512 * idx: 512 * (idx + 1)],
                                    krot[64 * h: 64 * h + 64,
                                         128 * t: 128 * t + 128],
                                    qrot[64 * h: 64 * h + 64,
                                         512 * b: 512 * (b + 1)],
                                    start=True, stop=True,
                                )
                            else:
                                rmin = 8 * jd
                                psv3 = ps[:, 512 * idx: 512 * (idx + 1)]\
                                    .rearrange("p (c r) -> p c r", r=32)
                                qv3 = qrot[64 * h: 64 * h + 64,
                                           512 * b: 512 * (b + 1)].rearrange(
                                    "p (c r) -> p c r", r=32)
                                nc.tensor.matmul(
                                    psv3[:, :, rmin:32],
                                    krot[64 * h: 64 * h + 64,
                                         128 * t: 128 * t + 128],
                                    qv3[:, :, rmin:32],
                                    start=True, stop=True,
                                )
                                nc.vector.tensor_add(
                                    psv3[:, :, rmin:rmin + 8],
                                    psv3[:, :, rmin:rmin + 8], tri3)
                        wtot = 512 * len(chunk)
                        nc.scalar.activation(pch[:, 0:wtot],
                                             ps[:, 0:wtot], EXP,
                                             scale=0.125)
                        emit_pv((chunk, pch, outp, opv3, b, nt))
                    if dbg and b == 0:
                        stg = ppool.tile([128, 512], f32, tag="dbgstg",
                                         name="dbgstg")
                        nc.vector.tensor_copy(stg[:], outp[0][:])
                        nc.sync.dma_start(dbg_aps["dbg_den0"][:], stg[:])
                    for h in range(HPC):
                        st = nrmp.tile([64, 1024], f32, tag="nrm")
                        nc.vector.tensor_copy(st[:, 0:512],
                                              outp[h][64:128, :])
                        nc.vector.reciprocal_approx_fast(
                            out=st[:, 512:1024], in_=st[:, 0:512])
                        nc.vector.tensor_mul(
                            outTs[h][:, 512 * b: 512 * (b + 1)],
                            outp[h][0:64, :], st[:, 512:1024])
                    if b == 3:
                        y_sb = big.tile([128, 4 * D], f32, tag="big",
                                        name="y_sb")
                        emit_ct(0, 0)
                    elif b == 4:
                        emit_ct(1, 0)
                if dbg:
                    nc.sync.dma_start(dbg_aps["dbg_outT0"][:], outTs[0][:])
                    nc.sync.dma_start(dbg_aps["dbg_outT1"][:], outTs[1][:])
                # ---------------- phase 4 (remaining) ----------------
                emit_ct(0, 1)
                emit_yproj(0, 1)
                emit_ct(1, 1)
                emit_yproj(1, 1)

    nc.compile()
    return nc


def kernel(**inputs):
    x = np.asarray(inputs["x"], dtype=np.float32)     # [1, 4096, 1024]
    Wq = np.asarray(inputs["Wq"], dtype=np.float32)
    Wk = np.asarray(inputs["Wk"], dtype=np.float32)
    Wv = np.asarray(inputs["Wv"], dtype=np.float32)
    Wo = np.asarray(inputs["Wo"], dtype=np.float32)
    for bn in ("bq", "bk", "bv", "bo"):
        bv_ = np.asarray(inputs.get(bn, 0.0))
        assert np.all(bv_ == 0.0), f"{bn} nonzero: unsupported"

    from concourse.bass_utils import run_bass_kernel_spmd

    dbg = bool(int(os.environ.get("BASS_KERNEL_DBG", "0")))
    key = ("nc", dbg)
    if key not in _CACHE:
        _CACHE[key] = _build_program(dbg)
    nc = _CACHE[key]

    bf = ml_dtypes.bfloat16
    wqT, wkT, wvT, woT, csq1, csq2, csk1, csk2, tri = _host_arrays(
        Wq, Wk, Wv, Wo)
    shared = {"wqT": wqT, "wkT": wkT, "wvT": wvT, "woT": woT,
              "csq1": csq1, "csq2": csq2, "csk1": csk1, "csk2": csk2,
              "tri": tri}
    xf = x.reshape(S, D)
    in_maps = []
    for i in range(NC_N):
        xTi = np.ascontiguousarray(xf[i * RPC:(i + 1) * RPC, :].T).astype(bf)
        in_maps.append(dict(shared, xT=xTi))

    trace = bool(int(os.environ.get("BASS_KERNEL_TRACE", "0")))
    res = run_bass_kernel_spmd(nc, in_maps, core_ids=list(range(NC_N)),
                               trace=trace)
    _CACHE["last_res"] = res
    if trace and res.exec_time_ns is not None:
        print(f"HW exec time: {res.exec_time_ns} ns")
        _CACHE["exec_time_ns"] = res.exec_time_ns
        _CACHE["trace"] = res.instructions_and_trace
    out = np.concatenate([res.results[i]["y"] for i in range(NC_N)], axis=0)
    return out.reshape(1, S, D).astype(np.float32)
